# revision 35
# baseline (speedup 1.0000x reference)
"""GQA attention (B=4, S=1024, H=4096, 32 q heads / 8 kv heads, rotary) on 8 trn2 cores.

Sharding: DP4 x TP2. Core c = 2*b + j handles batch b with kv-head half j:
  - column-parallel wq/wk/wv (16 q heads / 4 kv heads per core)
  - row-parallel wo -> partial [S, H] outputs, host sums core pairs.

All-bf16 matmul pipeline, single-pass projections with the full x resident
in SBUF (no staging DRAM round trips), v computed directly in natural [t, d]
layout (no PE transposes), host-packed DMA-contiguous weight blobs, and a
software pipeline that interleaves each q-head's scores matmuls into the
next head's projection so the ACT-engine exp stream (softmax) hides under PE
matmul work (scalar engine is the only exp engine; psc bank depth + a
one-head lag on the denominator/attn@v consumers keep the PE from stalling
on it).

Per-core dataflow:
  v[t, d] = xT.T @ wv streamed chunk-wise behind the x load (8 PSUM banks);
  kT[d, t] = wk.T @ xT + rope (channel-pair mix via host perm + partition
  swap DMA); per q-head block cb: qT = wq_cb.T @ xT + rope ->
    scoresT[t,s] = kT.T @ qT (interleaved into proj of cb+1); exp on ACT;
    denom via DVE tree-add over t-blocks + one ones-matmul partition sum
    (full [128,512] so the reciprocal runs on all DVE lanes);
    oT[d,s] = (v.T @ expT) * (1/denom)
  out = oT.T @ wo (bf16 operands, fp32 accumulate/out), host sums TP pairs.
"""

import numpy as np

B = 4
S = 1024
H = 4096
D = 128
HQ = 32
HKV = 8
G = 4
NCORES = 8
QC = 2048  # q cols per core
KC = 512  # k cols per core
VC = 512  # v cols per core
COH = 2048  # wo rows per core
ROPE_BASE = 10000.0

NKO = H // 128  # 32 contraction tiles
KOC = 8  # ko tiles per x chunk
NCH = NKO // KOC  # 4 chunks

_CACHE = {}


def _build(reps=1):
    import concourse.tile as tile
    from concourse import bacc, mybir

    fp32 = mybir.dt.float32
    bf16 = mybir.dt.bfloat16

    nc = bacc.Bacc(None, target_bir_lowering=False)

    if reps != 1:
        # shape depends on reps so each variant gets a distinct HLO hash
        # (the jax-level neff cache would otherwise reuse the reps=1 NEFF)
        nc.dram_tensor("repstag", [1, 16 * reps], fp32, kind="ExternalInput")

    # host-packed, per-partition-contiguous blobs (bf16)
    xp_d = nc.dram_tensor("xp", [NCH, 128, KOC, S], bf16, kind="ExternalInput")
    wqp_d = nc.dram_tensor("wqp", [16, 128, NKO, 128], bf16, kind="ExternalInput")
    wkp_d = nc.dram_tensor("wkp", [4, 128, NKO, 128], bf16, kind="ExternalInput")
    wvp_d = nc.dram_tensor("wvp", [NCH, 128, KOC, VC], bf16, kind="ExternalInput")
    wop_d = nc.dram_tensor("wop", [8, 2, 128, 8, 512], bf16, kind="ExternalInput")
    aq_d = nc.dram_tensor("ropeAq", [D, S], fp32, kind="ExternalInput")
    bq_d = nc.dram_tensor("ropeBq", [D, S], fp32, kind="ExternalInput")
    ak_d = nc.dram_tensor("ropeAk", [D, S], fp32, kind="ExternalInput")
    bk_d = nc.dram_tensor("ropeBk", [D, S], fp32, kind="ExternalInput")
    out_d = nc.dram_tensor("out", [S, H], fp32, kind="ExternalOutput")
    out_r = out_d.rearrange("(tb p) h -> tb p h", p=128)  # [8, 128, 4096]

    with tile.TileContext(nc) as tc, nc.allow_low_precision(
        reason="bf16 matmul pipeline"
    ):
      for _rep in range(reps):
        with (
            tc.tile_pool(name="persist", bufs=1) as persist,
            tc.tile_pool(name="konst", bufs=1) as konst,
        ):
            kT = persist.tile([128, HKV // 2, S], bf16)  # [128, 4, 1024]
            v = persist.tile([128, S // 128, VC], bf16)  # [128, 8, 512]
            fp16 = mybir.dt.float16
            ones_f = konst.tile([128, 128], fp32)
            nc.vector.memset(ones_f[:], 1.0)
            ones_h = konst.tile([128, 128], fp16)
            nc.vector.tensor_copy(ones_h[:], ones_f[:])

            with (
                tc.tile_pool(name="ot", bufs=1) as opool,
                tc.tile_pool(name="xt", bufs=1) as xpool,
            ):
                oT = opool.tile([128, 16, S], bf16)  # 32 KiB/part

                # Startup DMA discipline: every no-dep DMA otherwise fires at
                # t=0 in parallel, so the first x/wv chunks (which the first
                # v matmuls need) only land when the whole ~15 MiB initial
                # load does (~25 us PE startup stall). Chain the startup
                # stream in consumption order instead: wv0,x0,wv1,x1,...,
                # then rope maps, wk0/wk1, wq0.
                from concourse.tile_rust import add_dep_helper
                xts = []
                prev_level = []

                def chain(*dma_insts):
                    """One chain level: all depend on the whole prev level."""
                    insts = [getattr(d, "ins", d) for d in dma_insts]
                    for i in insts:
                        for prv in prev_level:
                            add_dep_helper(i, prv, sync=True,
                                           reason="startup DMA stream level")
                    prev_level[:] = insts

                for ch in range(NCH):
                    xt = xpool.tile([128, KOC, S], bf16, tag=f"xt{ch}",
                                    name=f"xt{ch}")
                    xts.append(xt)

                def rope_evict(epool, raw_ps, Am, Bm, out_ap, th):
                    """out = raw*Am + swap128(raw)*Bm  (raw in PSUM, fp32)."""
                    ts_ = slice(th * 512, th * 512 + 512)
                    raw = epool.tile([128, 512], fp32, tag="raw", name="raw")
                    nc.vector.tensor_copy(raw[:], raw_ps[:])
                    t1 = epool.tile([128, 512], fp32, tag="t1", name="t1")
                    nc.vector.tensor_mul(t1[:], raw_ps[:], Am[:, ts_])
                    sw = epool.tile([128, 512], fp32, tag="sw", name="sw")
                    nc.sync.dma_start(sw[0:64, :], raw[64:128, :])
                    nc.sync.dma_start(sw[64:128, :], raw[0:64, :])
                    t2 = epool.tile([128, 512], fp32, tag="t2", name="t2")
                    nc.vector.tensor_mul(t2[:], sw[:], Bm[:, ts_])
                    nc.vector.tensor_add(out_ap, t1[:], t2[:])

                def proj_block(wt, ps_pool, interleave=None, early=False):
                    """64 accumulating matmuls -> psA/psB [128, 512] fp32.

                    interleave: list of 0-arg closures (scores MMs) emitted
                    between contraction steps to pace the ACT exp stream.
                    early: start interleaving at once (first pipelined head,
                    when ACT is still idle) instead of steady-state pacing.
                    """
                    psA = ps_pool.tile([128, 512], fp32, tag="ps", name="psA")
                    psB = ps_pool.tile([128, 512], fp32, tag="ps", name="psB")
                    for ko in range(NKO):
                        xt = xts[ko // KOC]
                        j = ko % KOC
                        nc.tensor.matmul(
                            psA[:], wt[:, ko, :], xt[:, j, 0:512],
                            start=(ko == 0), stop=(ko == NKO - 1),
                        )
                        nc.tensor.matmul(
                            psB[:], wt[:, ko, :], xt[:, j, 512:1024],
                            start=(ko == 0), stop=(ko == NKO - 1),
                        )
                        if interleave and ((ko >= 11 and ko % 2 == 1)
                                           or (early and ko % 4 == 1)):
                            interleave.pop(0)()
                    return psA, psB

                # ---- v first (natural layout, 8 PSUM banks): its matmuls
                # stream chunk-by-chunk right behind the x DMAs, so the PE
                # starts ~3 us in instead of waiting for the whole x load ----
                with (
                    tc.tile_pool(name="mapsk", bufs=1) as mpoolk,
                    tc.tile_pool(name="wtk", bufs=2) as wpool,
                ):
                    def load_wk(cb):
                        wt = wpool.tile([128, NKO, 128], bf16, tag="wt",
                                        name=f"wk{cb}")
                        d = nc.sync.dma_start(wt[:], wkp_d[cb])
                        return wt, d

                    with (
                        tc.tile_pool(name="wtv", bufs=2) as wvpool,
                        tc.tile_pool(name="psv", bufs=8, space="PSUM") as psvpool,
                    ):
                        banks = [
                            psvpool.tile([128, VC], fp32, tag="psv",
                                         name=f"psv{tb}")
                            for tb in range(8)
                        ]
                        wvcs = []
                        # consumption-ordered startup stream: level ch is
                        # {wv_ch, x_ch} in parallel, after level ch-1.
                        # Chunk 0 is split into two half-levels so the first
                        # v matmuls start ~4 us earlier.
                        for ch in range(NCH):
                            wvc = wvpool.tile([128, KOC, VC], bf16, tag="wv",
                                              name=f"wv{ch}")
                            if ch == 0:
                                half = KOC // 2
                                chain(nc.sync.dma_start(
                                          wvc[:, 0:half, :],
                                          wvp_d[0][:, 0:half, :]),
                                      nc.sync.dma_start(
                                          xts[0][:, 0:half, :],
                                          xp_d[0][:, 0:half, :]))
                                chain(nc.sync.dma_start(
                                          wvc[:, half:KOC, :],
                                          wvp_d[0][:, half:KOC, :]),
                                      nc.sync.dma_start(
                                          xts[0][:, half:KOC, :],
                                          xp_d[0][:, half:KOC, :]))
                            else:
                                chain(nc.sync.dma_start(wvc[:], wvp_d[ch]),
                                      nc.sync.dma_start(xts[ch][:], xp_d[ch]))
                            wvcs.append(wvc)
                        for ch in range(NCH):
                            if ch < NCH - 1:
                                for j in range(KOC):
                                    for tb in range(8):
                                        nc.tensor.matmul(
                                            banks[tb][:],
                                            xts[ch][:, j,
                                                    tb * 128:(tb + 1) * 128],
                                            wvcs[ch][:, j, :],
                                            start=(ch == 0 and j == 0),
                                            stop=False,
                                        )
                            else:
                                # last chunk: tb-major so each bank finishes
                                # (and evicts) 8 matmuls before the next —
                                # spreads the 8 DVE evictions instead of
                                # bunching them after the final matmul
                                for tb in range(8):
                                    for j in range(KOC):
                                        nc.tensor.matmul(
                                            banks[tb][:],
                                            xts[ch][:, j,
                                                    tb * 128:(tb + 1) * 128],
                                            wvcs[ch][:, j, :],
                                            start=False,
                                            stop=(j == KOC - 1),
                                        )
                                    nc.vector.tensor_copy(v[:, tb, :],
                                                          banks[tb][:])

                    # ---- k projections (4 blocks) ----
                    mapk = {}
                    map_dmas = []
                    for nm, dram in (("Ak", ak_d), ("Bk", bk_d)):
                        mt = mpoolk.tile([128, S], fp32, tag=nm, name=nm)
                        map_dmas.append(nc.sync.dma_start(mt[:], dram[:]))
                        mapk[nm] = mt
                    with (
                        tc.tile_pool(name="evk", bufs=2) as epool,
                        tc.tile_pool(name="psk", bufs=3, space="PSUM") as pspool,
                    ):
                        wk0 = load_wk(0)
                        wk1 = load_wk(1)
                        chain(*map_dmas, wk0[1], wk1[1])
                        pending_wk = [wk0[0], wk1[0]]
                        for cb in range(4):
                            wt = pending_wk.pop(0)
                            psA, psB = proj_block(wt, pspool)
                            if cb + 2 < 4:
                                pending_wk.append(load_wk(cb + 2)[0])
                            for th, ps in ((0, psA), (1, psB)):
                                ts_ = slice(th * 512, th * 512 + 512)
                                rope_evict(epool, ps, mapk["Ak"], mapk["Bk"],
                                           kT[:, cb, ts_], th)

                # ---- q blocks softwarepipelined with attention ----
                with (
                    tc.tile_pool(name="mapsq", bufs=1) as mpoolq,
                    tc.tile_pool(name="wtq", bufs=2) as wqpool,
                    tc.tile_pool(name="evq", bufs=2) as epool,
                    tc.tile_pool(name="qbuf", bufs=2) as qpool,
                    tc.tile_pool(name="ex", bufs=2) as expool,
                    tc.tile_pool(name="sm", bufs=2) as smpool,
                    tc.tile_pool(name="psq", bufs=2, space="PSUM") as psqpool,
                    tc.tile_pool(name="pssc", bufs=4, space="PSUM") as pssc,
                    tc.tile_pool(name="psden", bufs=1, space="PSUM") as psden,
                    tc.tile_pool(name="pso", bufs=1, space="PSUM") as psopool,
                ):
                    mapq = {}
                    mq_dmas = []
                    for nm, dram in (("Aq", aq_d), ("Bq", bq_d)):
                        mt = mpoolq.tile([128, S], fp32, tag=nm, name=nm)
                        mq_dmas.append(nc.sync.dma_start(mt[:], dram[:]))
                        mapq[nm] = mt

                    wq_next = [None]
                    wq0_dma = []

                    def load_wq(cb):
                        wt = wqpool.tile([128, NKO, 128], bf16, tag="wt",
                                         name=f"wq{cb}")
                        d = nc.sync.dma_start(wt[:], wqp_d[cb])
                        if cb == 0:
                            wq0_dma.append(d)
                        return wt

                    wq_next[0] = load_wq(0)
                    chain(*mq_dmas, wq0_dma[0])

                    def make_scores(cb, qt):
                        """16 closures: scores MM + exp for (cb, sh, tb).
                        Returns (closures, expT tiles per sh)."""
                        h = cb // 4
                        exps = [
                            expool.tile([128, 8, 512], bf16, tag=f"expT{sh}",
                                        name=f"expT{cb}_{sh}")
                            for sh in range(2)
                        ]
                        closures = []
                        for sh in range(2):
                            ss = slice(sh * 512, sh * 512 + 512)
                            for tb in range(8):
                                def emit(sh=sh, ss=ss, tb=tb):
                                    psc = pssc.tile([128, 512], fp32,
                                                    tag="psc", name="psc")
                                    nc.tensor.matmul(
                                        psc[:],
                                        kT[:, h, tb * 128:(tb + 1) * 128],
                                        qt[:, ss],
                                        start=True, stop=True,
                                    )
                                    nc.scalar.activation(
                                        exps[sh][:, tb], psc[:],
                                        mybir.ActivationFunctionType.Exp,
                                    )
                                closures.append(emit)
                        return closures, exps

                    def emit_tails(cb, exps, interleave):
                        """softmax denom + attn@v for head-block cb."""
                        h = cb // 4
                        for sh in range(2):
                            ss = slice(sh * 512, sh * 512 + 512)
                            expT = exps[sh]
                            # denom: DVE tree-add over tb (saves 7 PE matmuls),
                            # then one ones-matmul for the partition sum
                            tt = []
                            for i in range(4):
                                t = smpool.tile([128, 512], fp16, tag=f"ta{i}",
                                                name=f"ta{i}")
                                nc.vector.tensor_add(t[:], expT[:, 2 * i],
                                                     expT[:, 2 * i + 1])
                                tt.append(t)
                                if interleave:
                                    interleave.pop(0)()
                            nc.vector.tensor_add(tt[0][:], tt[0][:], tt[1][:])
                            nc.vector.tensor_add(tt[2][:], tt[2][:], tt[3][:])
                            nc.vector.tensor_add(tt[0][:], tt[0][:], tt[2][:])
                            pden = psden.tile([128, 512], fp32, tag="pd",
                                              name="pd")
                            nc.tensor.matmul(pden[:], ones_h[:], tt[0][:],
                                             start=True, stop=True)
                            if interleave:
                                interleave.pop(0)()
                            invb = smpool.tile([128, 512], fp32, tag="invb",
                                               name="invb")
                            nc.vector.reciprocal_approx_fast(invb[:], pden[:])
                            po = psopool.tile([128, 512], fp32, tag="po",
                                              name="po")
                            for tb in range(8):
                                nc.tensor.matmul(
                                    po[:],
                                    v[:, tb, h * 128:(h + 1) * 128],
                                    expT[:, tb],
                                    start=(tb == 0), stop=(tb == 7),
                                )
                                if interleave and tb % 2 == 1:
                                    interleave.pop(0)()
                            nc.vector.tensor_mul(oT[:, cb, ss], po[:], invb[:])

                    pending_scores = []
                    pending_exps = None
                    for cb in range(16):
                        wt = wq_next[0]
                        psA, psB = proj_block(wt, psqpool,
                                              interleave=pending_scores,
                                              early=(cb == 1))
                        if cb + 1 < 16:
                            wq_next[0] = load_wq(cb + 1)
                        if cb == 14:
                            # prefetch first epilogue wo strip into the slot
                            # that proj(14) just freed
                            woe0 = wqpool.tile([128, 8, 512], bf16, tag="wt",
                                               name="woe0")
                            nc.sync.dma_start(woe0[:], wop_d[0, 0])
                        qt = qpool.tile([128, S], bf16, tag="qt", name="qt")
                        for th, ps in ((0, psA), (1, psB)):
                            ts_ = slice(th * 512, th * 512 + 512)
                            rope_evict(epool, ps, mapq["Aq"], mapq["Bq"],
                                       qt[:, ts_], th)
                        if cb > 0:
                            emit_tails(cb - 1, pending_exps, pending_scores)
                        assert not pending_scores
                        pending_scores, pending_exps = make_scores(cb, qt)

                    # epilogue: last head's scores, then two early phase-3
                    # output groups (hh=0, tb=0/1) on recycled pools fill the
                    # PE while ACT finishes the last head's exp stream; their
                    # co=15 step (needs oT[:,15] from tails(15)) comes after.
                    for c in pending_scores:
                        c()
                    woe1 = wqpool.tile([128, 8, 512], bf16, tag="wt",
                                       name="woe1")
                    nc.sync.dma_start(woe1[:], wop_d[0, 1])
                    woe = [woe0, woe1]
                    egs = [
                        psqpool.tile([128, 512], fp32, tag="ps", name=f"eg{tb}")
                        for tb in range(2)
                    ]
                    # co 0..7 (strip 0, already resident) for both groups
                    # while strip 1 loads; then co 8..14
                    for tb in range(2):
                        for co in range(8):
                            nc.tensor.matmul(
                                egs[tb][:], oT[:, co, tb * 128:(tb + 1) * 128],
                                woe[0][:, co, :],
                                start=(co == 0), stop=False,
                            )
                    for tb in range(2):
                        for co in range(8, 15):
                            nc.tensor.matmul(
                                egs[tb][:], oT[:, co, tb * 128:(tb + 1) * 128],
                                woe[1][:, co - 8, :],
                                start=False, stop=False,
                            )
                    emit_tails(15, pending_exps, [])
                    for tb in range(2):
                        nc.tensor.matmul(
                            egs[tb][:], oT[:, 15, tb * 128:(tb + 1) * 128],
                            woe[1][:, 7, :], start=False, stop=True,
                        )
                        et = epool.tile([128, 512], fp32, tag="t1", name="et")
                        nc.vector.tensor_copy(et[:], egs[tb][:])
                        nc.sync.dma_start(out_r[tb, :, 0:512], et[:])

                # ---- phase 3: out = oT.T @ wo ----
                # hh processed in pairs: each oT stationary-operand load
                # serves two matmuls (halves the LDWEIGHTS stream on HW).
                # wo strips live in the DEAD x tiles (xpool, outer scope):
                # their DMAs' WAR deps are proj(15)'s reads, so they start
                # ~13 us before the attention pools drain — no cold-start
                # stall waiting for freed SBUF addresses. Strip (hh, half)
                # of pair hp sits at xts[2*(hp%2) + (hh-2*hp)][:, 4*half:].
                with (
                    tc.tile_pool(name="outp", bufs=2) as outpool,
                    tc.tile_pool(name="psout", bufs=4, space="PSUM") as psout,
                ):
                    def load_strip_pair(hp):
                        TA = xts[2 * (hp % 2)]
                        TB = xts[2 * (hp % 2) + 1]
                        for half in range(2):
                            srcA = wop_d[2 * hp, half].rearrange(
                                "p (a b) c -> p a (b c)", b=2)
                            nc.sync.dma_start(
                                TA[:, 4 * half:4 * half + 4, :], srcA)
                            srcB = wop_d[2 * hp + 1, half].rearrange(
                                "p (a b) c -> p a (b c)", b=2)
                            nc.sync.dma_start(
                                TB[:, 4 * half:4 * half + 4, :], srcB)
                        return TA, TB

                    def strip_rhs(T, half, co8):
                        # strip flat offset co8*512 within the half's 4-ko
                        # region of the x tile
                        return T[:, 4 * half + co8 // 2,
                                 (co8 % 2) * 512:(co8 % 2) * 512 + 512]

                    tiles = {0: load_strip_pair(0), 1: load_strip_pair(1)}
                    for hp in range(4):
                        hh1 = 2 * hp + 1
                        TA, TB = tiles.pop(hp)
                        for tb in range(8):
                            # (hh=0, tb=0/1) were done early in the epilogue
                            skipA = hp == 0 and tb < 2
                            psoA = None if skipA else psout.tile(
                                [128, 512], fp32, tag="pso", name="psoA")
                            psoB = psout.tile([128, 512], fp32, tag="pso",
                                              name="psoB")
                            for co in range(16):
                                lhsT = oT[:, co, tb * 128:(tb + 1) * 128]
                                if not skipA:
                                    nc.tensor.matmul(
                                        psoA[:], lhsT,
                                        strip_rhs(TA, co // 8, co % 8),
                                        start=(co == 0), stop=(co == 15),
                                    )
                                nc.tensor.matmul(
                                    psoB[:], lhsT,
                                    strip_rhs(TB, co // 8, co % 8),
                                    start=(co == 0), stop=(co == 15),
                                )
                            targets = ((hh1, psoB),) if skipA else (
                                (2 * hp, psoA), (hh1, psoB))
                            for hh, pso_ in targets:
                                ot = outpool.tile([128, 512], fp32, tag="ot",
                                                  name="ot")
                                nc.vector.tensor_copy(ot[:], pso_[:])
                                nc.sync.dma_start(
                                    out_r[tb, :, hh * 512:(hh + 1) * 512],
                                    ot[:])
                        # prefetch pair hp+2 into the tiles hp just finished
                        # reading (emitted after all of hp's matmuls so the
                        # WAR ordering is correct)
                        if hp < 2:
                            tiles[hp + 2] = load_strip_pair(hp + 2)

    nc.compile()
    return nc


def _host_prep(x, wq, wk, wv, wo, start_pos):
    import ml_dtypes

    bf16 = ml_dtypes.bfloat16
    x = np.asarray(x, dtype=np.float32)
    wq = np.asarray(wq, dtype=np.float32)
    wk = np.asarray(wk, dtype=np.float32)
    wv = np.asarray(wv, dtype=np.float32)
    wo = np.asarray(wo, dtype=np.float32)
    sp = int(np.asarray(start_pos))

    perm = np.concatenate([np.arange(0, 128, 2), np.arange(1, 128, 2)])

    def pack_proj(w):
        # w: [H, C] -> [C/128, 128p, NKO, 128c] with rope perm on cols
        C = w.shape[1]
        r = w.reshape(NKO, 128, C // 128, 128)[:, :, :, perm]
        return np.ascontiguousarray(r.transpose(2, 1, 0, 3)).astype(bf16)

    def pack_v(w):
        # w: [H, VC] -> [NCH, 128p, KOC, VC] (no perm)
        r = w.reshape(NCH, KOC, 128, VC)
        return np.ascontiguousarray(r.transpose(0, 2, 1, 3)).astype(bf16)

    def pack_wo(w):
        # w: [COH, H] -> [8hh, 2half, 128p, 8co, 512]; wo row =
        # half*1024 + co*128 + p, col = hh*512 + c
        r = w.reshape(2, 8, 128, 8, 512)
        return np.ascontiguousarray(r.transpose(3, 0, 2, 1, 4)).astype(bf16)

    def pack_x(xb):
        # xb: [S, H] -> xT[H, S] -> [NCH, 128p, KOC, S]
        xT = np.ascontiguousarray(xb.T).reshape(NCH, KOC, 128, S)
        return np.ascontiguousarray(xT.transpose(0, 2, 1, 3)).astype(bf16)

    inv_freq = 1.0 / (ROPE_BASE ** (np.arange(0, D, 2, dtype=np.float32) / D))
    t = np.arange(sp, sp + S, dtype=np.float32)
    freqs = t[None, :] * inv_freq[:, None]  # [64, S]
    sin, cos = np.sin(freqs), np.cos(freqs)
    A = np.concatenate([sin, sin], axis=0).astype(np.float32)  # [128, S]
    Bm = np.concatenate([-cos, cos], axis=0).astype(np.float32)
    scale = np.float32(1.0 / np.sqrt(np.float32(D)))
    maps = {
        "ropeAq": np.ascontiguousarray(A * scale),
        "ropeBq": np.ascontiguousarray(Bm * scale),
        "ropeAk": np.ascontiguousarray(A),
        "ropeBk": np.ascontiguousarray(Bm),
    }

    # weights are shared across batches: pack once per tp half
    wpacks = []
    for j in range(2):
        wpacks.append({
            "wqp": pack_proj(wq[:, j * QC:(j + 1) * QC]),
            "wkp": pack_proj(wk[:, j * KC:(j + 1) * KC]),
            "wvp": pack_v(wv[:, j * VC:(j + 1) * VC]),
            "wop": pack_wo(wo[j * COH:(j + 1) * COH, :]),
        })
    xpacks = [pack_x(x[b]) for b in range(B)]

    in_maps = []
    for c in range(NCORES):
        b, j = divmod(c, 2)
        im = {"xp": xpacks[b]}
        im.update(wpacks[j])
        im.update(maps)
        in_maps.append(im)
    return in_maps


def kernel(x, wq, wk, wv, wo, start_pos=0, _trace=False):
    from concourse.bass_utils import run_bass_kernel_spmd

    if "nc" not in _CACHE:
        _CACHE["nc"] = _build()
    nc = _CACHE["nc"]

    in_maps = _host_prep(x, wq, wk, wv, wo, start_pos)
    res = run_bass_kernel_spmd(nc, in_maps, core_ids=list(range(NCORES)), trace=_trace)
    _CACHE["last_result"] = res

    out = np.empty((B, S, H), dtype=np.float32)
    for b in range(B):
        out[b] = res.results[2 * b]["out"] + res.results[2 * b + 1]["out"]
    return out


# revision 36
# speedup vs baseline: 1.0347x; 1.0347x over previous
"""GQA attention (B=4, S=1024, H=4096, 32 q heads / 8 kv heads, rotary) on 8 trn2 cores.

Sharding: DP4 x TP2. Core c = 2*b + j handles batch b with kv-head half j:
  - column-parallel wq/wk/wv (16 q heads / 4 kv heads per core)
  - row-parallel wo -> partial [S, H] outputs, host sums core pairs.

All-bf16 matmul pipeline, single-pass projections with the full x resident
in SBUF (no staging DRAM round trips), v computed directly in natural [t, d]
layout (no PE transposes), host-packed DMA-contiguous weight blobs, and a
software pipeline that interleaves each q-head's scores matmuls into the
next head's projection so the ACT-engine exp stream (softmax) hides under PE
matmul work (scalar engine is the only exp engine; psc bank depth + a
one-head lag on the denominator/attn@v consumers keep the PE from stalling
on it).

Per-core dataflow:
  v[t, d] = xT.T @ wv streamed chunk-wise behind the x load (8 PSUM banks);
  kT[d, t] = wk.T @ xT + rope (channel-pair mix via host perm + partition
  swap DMA); per q-head block cb: qT = wq_cb.T @ xT + rope ->
    scoresT[t,s] = kT.T @ qT (interleaved into proj of cb+1); exp on ACT;
    denom via DVE tree-add over t-blocks + one ones-matmul partition sum
    (full [128,512] so the reciprocal runs on all DVE lanes);
    oT[d,s] = (v.T @ expT) * (1/denom)
  out = oT.T @ wo (bf16 operands, fp32 accumulate/out), host sums TP pairs.
"""

import numpy as np

B = 4
S = 1024
H = 4096
D = 128
HQ = 32
HKV = 8
G = 4
NCORES = 8
QC = 2048  # q cols per core
KC = 512  # k cols per core
VC = 512  # v cols per core
COH = 2048  # wo rows per core
ROPE_BASE = 10000.0

NKO = H // 128  # 32 contraction tiles
KOC = 8  # ko tiles per x chunk
NCH = NKO // KOC  # 4 chunks

_CACHE = {}


def _build(reps=1):
    import concourse.tile as tile
    from concourse import bacc, mybir

    fp32 = mybir.dt.float32
    bf16 = mybir.dt.bfloat16

    nc = bacc.Bacc(None, target_bir_lowering=False)

    if reps != 1:
        # shape depends on reps so each variant gets a distinct HLO hash
        # (the jax-level neff cache would otherwise reuse the reps=1 NEFF)
        nc.dram_tensor("repstag", [1, 16 * reps], fp32, kind="ExternalInput")

    # host-packed, per-partition-contiguous blobs (bf16)
    xp_d = nc.dram_tensor("xp", [NCH, 128, KOC, S], bf16, kind="ExternalInput")
    wqp_d = nc.dram_tensor("wqp", [16, 128, NKO, 128], bf16, kind="ExternalInput")
    wkp_d = nc.dram_tensor("wkp", [4, 128, NKO, 128], bf16, kind="ExternalInput")
    wvp_d = nc.dram_tensor("wvp", [NCH, 128, KOC, VC], bf16, kind="ExternalInput")
    wop_d = nc.dram_tensor("wop", [8, 2, 128, 8, 512], bf16, kind="ExternalInput")
    aq_d = nc.dram_tensor("ropeAq", [D, S], fp32, kind="ExternalInput")
    bq_d = nc.dram_tensor("ropeBq", [D, S], fp32, kind="ExternalInput")
    ak_d = nc.dram_tensor("ropeAk", [D, S], fp32, kind="ExternalInput")
    bk_d = nc.dram_tensor("ropeBk", [D, S], fp32, kind="ExternalInput")
    out_d = nc.dram_tensor("out", [S, H], fp32, kind="ExternalOutput")
    out_r = out_d.rearrange("(tb p) h -> tb p h", p=128)  # [8, 128, 4096]

    with tile.TileContext(nc) as tc, nc.allow_low_precision(
        reason="bf16 matmul pipeline"
    ):
      for _rep in range(reps):
        with (
            tc.tile_pool(name="persist", bufs=1) as persist,
            tc.tile_pool(name="konst", bufs=1) as konst,
        ):
            kT = persist.tile([128, HKV // 2, S], bf16)  # [128, 4, 1024]
            v = persist.tile([128, S // 128, VC], bf16)  # [128, 8, 512]
            fp16 = mybir.dt.float16
            ones_f = konst.tile([128, 128], fp32)
            nc.vector.memset(ones_f[:], 1.0)
            ones_h = konst.tile([128, 128], fp16)
            nc.vector.tensor_copy(ones_h[:], ones_f[:])

            with (
                tc.tile_pool(name="ot", bufs=1) as opool,
                tc.tile_pool(name="xt", bufs=1) as xpool,
            ):
                oT = opool.tile([128, 16, S], bf16)  # 32 KiB/part

                # Startup DMA discipline: every no-dep DMA otherwise fires at
                # t=0 in parallel, so the first x/wv chunks (which the first
                # v matmuls need) only land when the whole ~15 MiB initial
                # load does (~25 us PE startup stall). Chain the startup
                # stream in consumption order instead: wv0,x0,wv1,x1,...,
                # then rope maps, wk0/wk1, wq0.
                from concourse.tile_rust import add_dep_helper
                xts = []
                prev_level = []

                def chain(*dma_insts):
                    """One chain level: all depend on the whole prev level."""
                    insts = [getattr(d, "ins", d) for d in dma_insts]
                    for i in insts:
                        for prv in prev_level:
                            add_dep_helper(i, prv, sync=True,
                                           reason="startup DMA stream level")
                    prev_level[:] = insts

                for ch in range(NCH):
                    xt = xpool.tile([128, KOC, S], bf16, tag=f"xt{ch}",
                                    name=f"xt{ch}")
                    xts.append(xt)

                def rope_evict(epool, raw_ps, Am, Bm, out_ap, th):
                    """out = raw*Am + swap128(raw)*Bm  (raw in PSUM, fp32)."""
                    ts_ = slice(th * 512, th * 512 + 512)
                    raw = epool.tile([128, 512], fp32, tag="raw", name="raw")
                    nc.vector.tensor_copy(raw[:], raw_ps[:])
                    t1 = epool.tile([128, 512], fp32, tag="t1", name="t1")
                    nc.vector.tensor_mul(t1[:], raw_ps[:], Am[:, ts_])
                    sw = epool.tile([128, 512], fp32, tag="sw", name="sw")
                    nc.sync.dma_start(sw[0:64, :], raw[64:128, :])
                    nc.sync.dma_start(sw[64:128, :], raw[0:64, :])
                    t2 = epool.tile([128, 512], fp32, tag="t2", name="t2")
                    nc.vector.tensor_mul(t2[:], sw[:], Bm[:, ts_])
                    nc.vector.tensor_add(out_ap, t1[:], t2[:])

                def proj_block(wt, ps_pool, interleave=None, early=False):
                    """64 accumulating matmuls -> psA/psB [128, 512] fp32.

                    interleave: list of 0-arg closures (scores MMs) emitted
                    between contraction steps to pace the ACT exp stream.
                    early: start interleaving at once (first pipelined head,
                    when ACT is still idle) instead of steady-state pacing.
                    """
                    psA = ps_pool.tile([128, 512], fp32, tag="ps", name="psA")
                    psB = ps_pool.tile([128, 512], fp32, tag="ps", name="psB")
                    for ko in range(NKO):
                        xt = xts[ko // KOC]
                        j = ko % KOC
                        nc.tensor.matmul(
                            psA[:], wt[:, ko, :], xt[:, j, 0:512],
                            start=(ko == 0), stop=(ko == NKO - 1),
                        )
                        nc.tensor.matmul(
                            psB[:], wt[:, ko, :], xt[:, j, 512:1024],
                            start=(ko == 0), stop=(ko == NKO - 1),
                        )
                        if interleave and ((ko >= 11 and ko % 2 == 1)
                                           or (early and ko % 4 == 1)):
                            interleave.pop(0)()
                    return psA, psB

                # ---- v first (natural layout, 8 PSUM banks): its matmuls
                # stream chunk-by-chunk right behind the x DMAs, so the PE
                # starts ~3 us in instead of waiting for the whole x load ----
                with (
                    tc.tile_pool(name="mapsk", bufs=1) as mpoolk,
                    tc.tile_pool(name="wtk", bufs=2) as wpool,
                ):
                    def load_wk(cb):
                        wt = wpool.tile([128, NKO, 128], bf16, tag="wt",
                                        name=f"wk{cb}")
                        d = nc.sync.dma_start(wt[:], wkp_d[cb])
                        return wt, d

                    with (
                        tc.tile_pool(name="wtv", bufs=2) as wvpool,
                        tc.tile_pool(name="psv", bufs=8, space="PSUM") as psvpool,
                    ):
                        banks = [
                            psvpool.tile([128, VC], fp32, tag="psv",
                                         name=f"psv{tb}")
                            for tb in range(8)
                        ]
                        wvcs = []
                        # consumption-ordered startup stream: level ch is
                        # {wv_ch, x_ch} in parallel, after level ch-1.
                        # Chunk 0 is split into two half-levels so the first
                        # v matmuls start ~4 us earlier.
                        for ch in range(NCH):
                            wvc = wvpool.tile([128, KOC, VC], bf16, tag="wv",
                                              name=f"wv{ch}")
                            if ch == 0:
                                # quarter-granularity levels: the PE's first
                                # matmuls start behind a 0.6 MiB bite
                                for q in range(4):
                                    qs = slice(2 * q, 2 * q + 2)
                                    chain(nc.sync.dma_start(
                                              wvc[:, qs, :],
                                              wvp_d[0][:, qs, :]),
                                          nc.sync.dma_start(
                                              xts[0][:, qs, :],
                                              xp_d[0][:, qs, :]))
                            else:
                                chain(nc.sync.dma_start(wvc[:], wvp_d[ch]),
                                      nc.sync.dma_start(xts[ch][:], xp_d[ch]))
                            wvcs.append(wvc)
                        for ch in range(NCH):
                            if ch < NCH - 1:
                                for j in range(KOC):
                                    for tb in range(8):
                                        nc.tensor.matmul(
                                            banks[tb][:],
                                            xts[ch][:, j,
                                                    tb * 128:(tb + 1) * 128],
                                            wvcs[ch][:, j, :],
                                            start=(ch == 0 and j == 0),
                                            stop=False,
                                        )
                            else:
                                # last chunk: tb-major so each bank finishes
                                # (and evicts) 8 matmuls before the next —
                                # spreads the 8 DVE evictions instead of
                                # bunching them after the final matmul
                                for tb in range(8):
                                    for j in range(KOC):
                                        nc.tensor.matmul(
                                            banks[tb][:],
                                            xts[ch][:, j,
                                                    tb * 128:(tb + 1) * 128],
                                            wvcs[ch][:, j, :],
                                            start=False,
                                            stop=(j == KOC - 1),
                                        )
                                    nc.vector.tensor_copy(v[:, tb, :],
                                                          banks[tb][:])

                    # ---- k projections (4 blocks) ----
                    mapk = {}
                    map_dmas = []
                    for nm, dram in (("Ak", ak_d), ("Bk", bk_d)):
                        mt = mpoolk.tile([128, S], fp32, tag=nm, name=nm)
                        map_dmas.append(nc.sync.dma_start(mt[:], dram[:]))
                        mapk[nm] = mt
                    with (
                        tc.tile_pool(name="evk", bufs=2) as epool,
                        tc.tile_pool(name="psk", bufs=3, space="PSUM") as pspool,
                    ):
                        wk0 = load_wk(0)
                        wk1 = load_wk(1)
                        chain(*map_dmas, wk0[1], wk1[1])
                        pending_wk = [wk0[0], wk1[0]]
                        for cb in range(4):
                            wt = pending_wk.pop(0)
                            psA, psB = proj_block(wt, pspool)
                            if cb + 2 < 4:
                                pending_wk.append(load_wk(cb + 2)[0])
                            for th, ps in ((0, psA), (1, psB)):
                                ts_ = slice(th * 512, th * 512 + 512)
                                rope_evict(epool, ps, mapk["Ak"], mapk["Bk"],
                                           kT[:, cb, ts_], th)

                # ---- q blocks softwarepipelined with attention ----
                with (
                    tc.tile_pool(name="mapsq", bufs=1) as mpoolq,
                    tc.tile_pool(name="wtq", bufs=2) as wqpool,
                    tc.tile_pool(name="evq", bufs=2) as epool,
                    tc.tile_pool(name="qbuf", bufs=2) as qpool,
                    tc.tile_pool(name="ex", bufs=2) as expool,
                    tc.tile_pool(name="sm", bufs=2) as smpool,
                    tc.tile_pool(name="psq", bufs=2, space="PSUM") as psqpool,
                    tc.tile_pool(name="pssc", bufs=4, space="PSUM") as pssc,
                    tc.tile_pool(name="psden", bufs=1, space="PSUM") as psden,
                    tc.tile_pool(name="pso", bufs=1, space="PSUM") as psopool,
                ):
                    mapq = {}
                    mq_dmas = []
                    for nm, dram in (("Aq", aq_d), ("Bq", bq_d)):
                        mt = mpoolq.tile([128, S], fp32, tag=nm, name=nm)
                        mq_dmas.append(nc.sync.dma_start(mt[:], dram[:]))
                        mapq[nm] = mt

                    wq_next = [None]
                    wq0_dma = []

                    def load_wq(cb):
                        wt = wqpool.tile([128, NKO, 128], bf16, tag="wt",
                                         name=f"wq{cb}")
                        d = nc.sync.dma_start(wt[:], wqp_d[cb])
                        if cb == 0:
                            wq0_dma.append(d)
                        return wt

                    wq_next[0] = load_wq(0)
                    chain(*mq_dmas, wq0_dma[0])

                    def make_scores(cb, qt):
                        """16 closures: scores MM + exp for (cb, sh, tb).
                        Returns (closures, expT tiles per sh)."""
                        h = cb // 4
                        exps = [
                            expool.tile([128, 8, 512], bf16, tag=f"expT{sh}",
                                        name=f"expT{cb}_{sh}")
                            for sh in range(2)
                        ]
                        closures = []
                        for sh in range(2):
                            ss = slice(sh * 512, sh * 512 + 512)
                            for tb in range(8):
                                def emit(sh=sh, ss=ss, tb=tb):
                                    psc = pssc.tile([128, 512], fp32,
                                                    tag="psc", name="psc")
                                    nc.tensor.matmul(
                                        psc[:],
                                        kT[:, h, tb * 128:(tb + 1) * 128],
                                        qt[:, ss],
                                        start=True, stop=True,
                                    )
                                    nc.scalar.activation(
                                        exps[sh][:, tb], psc[:],
                                        mybir.ActivationFunctionType.Exp,
                                    )
                                closures.append(emit)
                        return closures, exps

                    def emit_tails(cb, exps, interleave):
                        """softmax denom + attn@v for head-block cb."""
                        h = cb // 4
                        for sh in range(2):
                            ss = slice(sh * 512, sh * 512 + 512)
                            expT = exps[sh]
                            # denom: DVE tree-add over tb (saves 7 PE matmuls),
                            # then one ones-matmul for the partition sum
                            tt = []
                            for i in range(4):
                                t = smpool.tile([128, 512], fp16, tag=f"ta{i}",
                                                name=f"ta{i}")
                                nc.vector.tensor_add(t[:], expT[:, 2 * i],
                                                     expT[:, 2 * i + 1])
                                tt.append(t)
                                if interleave:
                                    interleave.pop(0)()
                            nc.vector.tensor_add(tt[0][:], tt[0][:], tt[1][:])
                            nc.vector.tensor_add(tt[2][:], tt[2][:], tt[3][:])
                            nc.vector.tensor_add(tt[0][:], tt[0][:], tt[2][:])
                            pden = psden.tile([128, 512], fp32, tag="pd",
                                              name="pd")
                            nc.tensor.matmul(pden[:], ones_h[:], tt[0][:],
                                             start=True, stop=True)
                            if interleave:
                                interleave.pop(0)()
                            invb = smpool.tile([128, 512], fp32, tag="invb",
                                               name="invb")
                            nc.vector.reciprocal_approx_fast(invb[:], pden[:])
                            po = psopool.tile([128, 512], fp32, tag="po",
                                              name="po")
                            for tb in range(8):
                                nc.tensor.matmul(
                                    po[:],
                                    v[:, tb, h * 128:(h + 1) * 128],
                                    expT[:, tb],
                                    start=(tb == 0), stop=(tb == 7),
                                )
                                if interleave and tb % 2 == 1:
                                    interleave.pop(0)()
                            nc.vector.tensor_mul(oT[:, cb, ss], po[:], invb[:])

                    pending_scores = []
                    pending_exps = None
                    for cb in range(16):
                        wt = wq_next[0]
                        psA, psB = proj_block(wt, psqpool,
                                              interleave=pending_scores,
                                              early=(cb == 1))
                        if cb + 1 < 16:
                            wq_next[0] = load_wq(cb + 1)
                        if cb == 14:
                            # prefetch first epilogue wo strip into the slot
                            # that proj(14) just freed
                            woe0 = wqpool.tile([128, 8, 512], bf16, tag="wt",
                                               name="woe0")
                            nc.sync.dma_start(woe0[:], wop_d[0, 0])
                        qt = qpool.tile([128, S], bf16, tag="qt", name="qt")
                        for th, ps in ((0, psA), (1, psB)):
                            ts_ = slice(th * 512, th * 512 + 512)
                            rope_evict(epool, ps, mapq["Aq"], mapq["Bq"],
                                       qt[:, ts_], th)
                        if cb > 0:
                            emit_tails(cb - 1, pending_exps, pending_scores)
                        assert not pending_scores
                        pending_scores, pending_exps = make_scores(cb, qt)

                    # epilogue: last head's scores, then two early phase-3
                    # output groups (hh=0, tb=0/1) on recycled pools fill the
                    # PE while ACT finishes the last head's exp stream; their
                    # co=15 step (needs oT[:,15] from tails(15)) comes after.
                    for c in pending_scores:
                        c()
                    woe1 = wqpool.tile([128, 8, 512], bf16, tag="wt",
                                       name="woe1")
                    nc.sync.dma_start(woe1[:], wop_d[0, 1])
                    woe = [woe0, woe1]
                    egs = [
                        psqpool.tile([128, 512], fp32, tag="ps", name=f"eg{tb}")
                        for tb in range(2)
                    ]
                    # co 0..7 (strip 0, already resident) for both groups
                    # while strip 1 loads; then co 8..14
                    for tb in range(2):
                        for co in range(8):
                            nc.tensor.matmul(
                                egs[tb][:], oT[:, co, tb * 128:(tb + 1) * 128],
                                woe[0][:, co, :],
                                start=(co == 0), stop=False,
                            )
                    for tb in range(2):
                        for co in range(8, 15):
                            nc.tensor.matmul(
                                egs[tb][:], oT[:, co, tb * 128:(tb + 1) * 128],
                                woe[1][:, co - 8, :],
                                start=False, stop=False,
                            )
                    emit_tails(15, pending_exps, [])
                    for tb in range(2):
                        nc.tensor.matmul(
                            egs[tb][:], oT[:, 15, tb * 128:(tb + 1) * 128],
                            woe[1][:, 7, :], start=False, stop=True,
                        )
                        et = epool.tile([128, 512], fp32, tag="t1", name="et")
                        nc.vector.tensor_copy(et[:], egs[tb][:])
                        nc.sync.dma_start(out_r[tb, :, 0:512], et[:])

                # ---- phase 3: out = oT.T @ wo ----
                # hh processed in pairs: each oT stationary-operand load
                # serves two matmuls (halves the LDWEIGHTS stream on HW).
                # wo strips live in the DEAD x tiles (xpool, outer scope):
                # their DMAs' WAR deps are proj(15)'s reads, so they start
                # ~13 us before the attention pools drain — no cold-start
                # stall waiting for freed SBUF addresses. Strip (hh, half)
                # of pair hp sits at xts[2*(hp%2) + (hh-2*hp)][:, 4*half:].
                with (
                    tc.tile_pool(name="outp", bufs=2) as outpool,
                    tc.tile_pool(name="psout", bufs=4, space="PSUM") as psout,
                ):
                    def load_strip_pair(hp):
                        TA = xts[2 * (hp % 2)]
                        TB = xts[2 * (hp % 2) + 1]
                        for half in range(2):
                            srcA = wop_d[2 * hp, half].rearrange(
                                "p (a b) c -> p a (b c)", b=2)
                            nc.sync.dma_start(
                                TA[:, 4 * half:4 * half + 4, :], srcA)
                            srcB = wop_d[2 * hp + 1, half].rearrange(
                                "p (a b) c -> p a (b c)", b=2)
                            nc.sync.dma_start(
                                TB[:, 4 * half:4 * half + 4, :], srcB)
                        return TA, TB

                    def strip_rhs(T, half, co8):
                        # strip flat offset co8*512 within the half's 4-ko
                        # region of the x tile
                        return T[:, 4 * half + co8 // 2,
                                 (co8 % 2) * 512:(co8 % 2) * 512 + 512]

                    tiles = {0: load_strip_pair(0), 1: load_strip_pair(1)}
                    for hp in range(4):
                        hh1 = 2 * hp + 1
                        TA, TB = tiles.pop(hp)
                        for tb in range(8):
                            # (hh=0, tb=0/1) were done early in the epilogue
                            skipA = hp == 0 and tb < 2
                            psoA = None if skipA else psout.tile(
                                [128, 512], fp32, tag="pso", name="psoA")
                            psoB = psout.tile([128, 512], fp32, tag="pso",
                                              name="psoB")
                            for co in range(16):
                                lhsT = oT[:, co, tb * 128:(tb + 1) * 128]
                                if not skipA:
                                    nc.tensor.matmul(
                                        psoA[:], lhsT,
                                        strip_rhs(TA, co // 8, co % 8),
                                        start=(co == 0), stop=(co == 15),
                                    )
                                nc.tensor.matmul(
                                    psoB[:], lhsT,
                                    strip_rhs(TB, co // 8, co % 8),
                                    start=(co == 0), stop=(co == 15),
                                )
                            targets = ((hh1, psoB),) if skipA else (
                                (2 * hp, psoA), (hh1, psoB))
                            for hh, pso_ in targets:
                                ot = outpool.tile([128, 512], fp32, tag="ot",
                                                  name="ot")
                                nc.vector.tensor_copy(ot[:], pso_[:])
                                nc.sync.dma_start(
                                    out_r[tb, :, hh * 512:(hh + 1) * 512],
                                    ot[:])
                        # prefetch pair hp+2 into the tiles hp just finished
                        # reading (emitted after all of hp's matmuls so the
                        # WAR ordering is correct)
                        if hp < 2:
                            tiles[hp + 2] = load_strip_pair(hp + 2)

    nc.compile()
    return nc


def _host_prep(x, wq, wk, wv, wo, start_pos):
    import ml_dtypes

    bf16 = ml_dtypes.bfloat16
    x = np.asarray(x, dtype=np.float32)
    wq = np.asarray(wq, dtype=np.float32)
    wk = np.asarray(wk, dtype=np.float32)
    wv = np.asarray(wv, dtype=np.float32)
    wo = np.asarray(wo, dtype=np.float32)
    sp = int(np.asarray(start_pos))

    perm = np.concatenate([np.arange(0, 128, 2), np.arange(1, 128, 2)])

    def pack_proj(w):
        # w: [H, C] -> [C/128, 128p, NKO, 128c] with rope perm on cols
        C = w.shape[1]
        r = w.reshape(NKO, 128, C // 128, 128)[:, :, :, perm]
        return np.ascontiguousarray(r.transpose(2, 1, 0, 3)).astype(bf16)

    def pack_v(w):
        # w: [H, VC] -> [NCH, 128p, KOC, VC] (no perm)
        r = w.reshape(NCH, KOC, 128, VC)
        return np.ascontiguousarray(r.transpose(0, 2, 1, 3)).astype(bf16)

    def pack_wo(w):
        # w: [COH, H] -> [8hh, 2half, 128p, 8co, 512]; wo row =
        # half*1024 + co*128 + p, col = hh*512 + c
        r = w.reshape(2, 8, 128, 8, 512)
        return np.ascontiguousarray(r.transpose(3, 0, 2, 1, 4)).astype(bf16)

    def pack_x(xb):
        # xb: [S, H] -> xT[H, S] -> [NCH, 128p, KOC, S]
        xT = np.ascontiguousarray(xb.T).reshape(NCH, KOC, 128, S)
        return np.ascontiguousarray(xT.transpose(0, 2, 1, 3)).astype(bf16)

    inv_freq = 1.0 / (ROPE_BASE ** (np.arange(0, D, 2, dtype=np.float32) / D))
    t = np.arange(sp, sp + S, dtype=np.float32)
    freqs = t[None, :] * inv_freq[:, None]  # [64, S]
    sin, cos = np.sin(freqs), np.cos(freqs)
    A = np.concatenate([sin, sin], axis=0).astype(np.float32)  # [128, S]
    Bm = np.concatenate([-cos, cos], axis=0).astype(np.float32)
    scale = np.float32(1.0 / np.sqrt(np.float32(D)))
    maps = {
        "ropeAq": np.ascontiguousarray(A * scale),
        "ropeBq": np.ascontiguousarray(Bm * scale),
        "ropeAk": np.ascontiguousarray(A),
        "ropeBk": np.ascontiguousarray(Bm),
    }

    # weights are shared across batches: pack once per tp half
    wpacks = []
    for j in range(2):
        wpacks.append({
            "wqp": pack_proj(wq[:, j * QC:(j + 1) * QC]),
            "wkp": pack_proj(wk[:, j * KC:(j + 1) * KC]),
            "wvp": pack_v(wv[:, j * VC:(j + 1) * VC]),
            "wop": pack_wo(wo[j * COH:(j + 1) * COH, :]),
        })
    xpacks = [pack_x(x[b]) for b in range(B)]

    in_maps = []
    for c in range(NCORES):
        b, j = divmod(c, 2)
        im = {"xp": xpacks[b]}
        im.update(wpacks[j])
        im.update(maps)
        in_maps.append(im)
    return in_maps


def kernel(x, wq, wk, wv, wo, start_pos=0, _trace=False):
    from concourse.bass_utils import run_bass_kernel_spmd

    if "nc" not in _CACHE:
        _CACHE["nc"] = _build()
    nc = _CACHE["nc"]

    in_maps = _host_prep(x, wq, wk, wv, wo, start_pos)
    res = run_bass_kernel_spmd(nc, in_maps, core_ids=list(range(NCORES)), trace=_trace)
    _CACHE["last_result"] = res

    out = np.empty((B, S, H), dtype=np.float32)
    for b in range(B):
        out[b] = res.results[2 * b]["out"] + res.results[2 * b + 1]["out"]
    return out


# revision 39
# speedup vs baseline: 1.1577x; 1.1189x over previous
"""GQA attention (B=4, S=1024, H=4096, 32 q heads / 8 kv heads, rotary) on 8 trn2 cores.

Sharding: DP4 x TP2. Core c = 2*b + j handles batch b with kv-head half j:
  - column-parallel wq/wk/wv (16 q heads / 4 kv heads per core)
  - row-parallel wo -> partial [S, H] outputs, host sums core pairs.

All-bf16 matmul pipeline, single-pass projections with the full x resident
in SBUF (no staging DRAM round trips), v computed directly in natural [t, d]
layout (no PE transposes), host-packed DMA-contiguous weight blobs, and a
software pipeline that interleaves each q-head's scores matmuls into the
next head's projection so the ACT-engine exp stream (softmax) hides under PE
matmul work (scalar engine is the only exp engine; psc bank depth + a
one-head lag on the denominator/attn@v consumers keep the PE from stalling
on it).

Per-core dataflow:
  v[t, d] = xT.T @ wv streamed chunk-wise behind the x load (8 PSUM banks);
  kT[d, t] = wk.T @ xT + rope (channel-pair mix via host perm + partition
  swap DMA); per q-head block cb: qT = wq_cb.T @ xT + rope ->
    scoresT[t,s] = kT.T @ qT (interleaved into proj of cb+1); exp on ACT;
    denom via DVE tree-add over t-blocks + one ones-matmul partition sum
    (full [128,512] so the reciprocal runs on all DVE lanes);
    oT[d,s] = (v.T @ expT) * (1/denom)
  out = oT.T @ wo (bf16 operands, fp32 accumulate/out), host sums TP pairs.
"""

import numpy as np

B = 4
S = 1024
H = 4096
D = 128
HQ = 32
HKV = 8
G = 4
NCORES = 8
QC = 2048  # q cols per core
KC = 512  # k cols per core
VC = 512  # v cols per core
COH = 2048  # wo rows per core
ROPE_BASE = 10000.0

NKO = H // 128  # 32 contraction tiles
KOC = 8  # ko tiles per x chunk
NCH = NKO // KOC  # 4 chunks

_CACHE = {}


def _build(reps=1):
    import concourse.tile as tile
    from concourse import bacc, mybir

    fp32 = mybir.dt.float32
    bf16 = mybir.dt.bfloat16

    nc = bacc.Bacc(None, target_bir_lowering=False)

    if reps != 1:
        # shape depends on reps so each variant gets a distinct HLO hash
        # (the jax-level neff cache would otherwise reuse the reps=1 NEFF)
        nc.dram_tensor("repstag", [1, 16 * reps], fp32, kind="ExternalInput")

    # host-packed, per-partition-contiguous blobs (bf16)
    xp_d = nc.dram_tensor("xp", [NCH, 128, KOC, S], bf16, kind="ExternalInput")
    wqp_d = nc.dram_tensor("wqp", [16, 128, NKO, 128], bf16, kind="ExternalInput")
    wkp_d = nc.dram_tensor("wkp", [4, 128, NKO, 128], bf16, kind="ExternalInput")
    wvp_d = nc.dram_tensor("wvp", [NCH, 128, KOC, VC], bf16, kind="ExternalInput")
    wop_d = nc.dram_tensor("wop", [8, 2, 128, 8, 512], bf16, kind="ExternalInput")
    aq_d = nc.dram_tensor("ropeAq", [D, S], fp32, kind="ExternalInput")
    bq_d = nc.dram_tensor("ropeBq", [D, S], fp32, kind="ExternalInput")
    ak_d = nc.dram_tensor("ropeAk", [D, S], fp32, kind="ExternalInput")
    bk_d = nc.dram_tensor("ropeBk", [D, S], fp32, kind="ExternalInput")
    out_d = nc.dram_tensor("out", [S, H], fp32, kind="ExternalOutput")
    out_r = out_d.rearrange("(tb p) h -> tb p h", p=128)  # [8, 128, 4096]

    with tile.TileContext(nc) as tc, nc.allow_low_precision(
        reason="bf16 matmul pipeline"
    ):
      for _rep in range(reps):
        with (
            tc.tile_pool(name="persist", bufs=1) as persist,
            tc.tile_pool(name="konst", bufs=1) as konst,
        ):
            kT = persist.tile([128, HKV // 2, S], bf16)  # [128, 4, 1024]
            v = persist.tile([128, S // 128, VC], bf16)  # [128, 8, 512]
            fp16 = mybir.dt.float16
            ones_f = konst.tile([128, 128], fp32)
            nc.vector.memset(ones_f[:], 1.0)
            ones_h = konst.tile([128, 128], fp16)
            nc.vector.tensor_copy(ones_h[:], ones_f[:])
            # warm the ACT Exp table now (idle engine) so the first real
            # softmax exp doesn't pay the 1.3 us LoadActFuncSet mid-pipeline
            actwarm = konst.tile([1, 2], fp32)
            nc.scalar.activation(actwarm[:], ones_f[0:1, 0:2],
                                 mybir.ActivationFunctionType.Exp)

            with (
                tc.tile_pool(name="ot", bufs=1) as opool,
                tc.tile_pool(name="xt", bufs=1) as xpool,
            ):
                oT = opool.tile([128, 16, S], bf16)  # 32 KiB/part

                # Startup DMA discipline: every no-dep DMA otherwise fires at
                # t=0 in parallel, so the first x/wv chunks (which the first
                # v matmuls need) only land when the whole ~15 MiB initial
                # load does (~25 us PE startup stall). Chain the startup
                # stream in consumption order instead: wv0,x0,wv1,x1,...,
                # then rope maps, wk0/wk1, wq0.
                from concourse.tile_rust import add_dep_helper
                xts = []
                prev_level = []

                def chain(*dma_insts):
                    """One chain level: all depend on the whole prev level."""
                    insts = [getattr(d, "ins", d) for d in dma_insts]
                    for i in insts:
                        for prv in prev_level:
                            add_dep_helper(i, prv, sync=True,
                                           reason="startup DMA stream level")
                    prev_level[:] = insts

                for ch in range(NCH):
                    xt = xpool.tile([128, KOC, S], bf16, tag=f"xt{ch}",
                                    name=f"xt{ch}")
                    xts.append(xt)

                def rope_evict(epool, raw_ps, Am, Bm, out_ap, th):
                    """out = raw*Am + swap128(raw)*Bm  (raw in PSUM, fp32)."""
                    ts_ = slice(th * 512, th * 512 + 512)
                    raw = epool.tile([128, 512], fp32, tag="raw", name="raw")
                    nc.vector.tensor_copy(raw[:], raw_ps[:])
                    t1 = epool.tile([128, 512], fp32, tag="t1", name="t1")
                    nc.vector.tensor_mul(t1[:], raw_ps[:], Am[:, ts_])
                    sw = epool.tile([128, 512], fp32, tag="sw", name="sw")
                    nc.sync.dma_start(sw[0:64, :], raw[64:128, :])
                    nc.sync.dma_start(sw[64:128, :], raw[0:64, :])
                    t2 = epool.tile([128, 512], fp32, tag="t2", name="t2")
                    nc.vector.tensor_mul(t2[:], sw[:], Bm[:, ts_])
                    nc.vector.tensor_add(out_ap, t1[:], t2[:])

                def proj_block(wt, ps_pool, interleave=None, early=False):
                    """64 accumulating matmuls -> psA/psB [128, 512] fp32.

                    interleave: list of 0-arg closures (scores MMs) emitted
                    between contraction steps to pace the ACT exp stream.
                    early: start interleaving at once (first pipelined head,
                    when ACT is still idle) instead of steady-state pacing.
                    """
                    psA = ps_pool.tile([128, 512], fp32, tag="ps", name="psA")
                    psB = ps_pool.tile([128, 512], fp32, tag="ps", name="psB")
                    for ko in range(NKO):
                        xt = xts[ko // KOC]
                        j = ko % KOC
                        nc.tensor.matmul(
                            psA[:], wt[:, ko, :], xt[:, j, 0:512],
                            start=(ko == 0), stop=(ko == NKO - 1),
                        )
                        nc.tensor.matmul(
                            psB[:], wt[:, ko, :], xt[:, j, 512:1024],
                            start=(ko == 0), stop=(ko == NKO - 1),
                        )
                        if interleave and ((ko >= 11 and ko % 2 == 1)
                                           or (early and ko % 4 == 1)):
                            interleave.pop(0)()
                    return psA, psB

                # ---- v first (natural layout, 8 PSUM banks): its matmuls
                # stream chunk-by-chunk right behind the x DMAs, so the PE
                # starts ~3 us in instead of waiting for the whole x load ----
                with (
                    tc.tile_pool(name="mapsk", bufs=1) as mpoolk,
                    tc.tile_pool(name="wtk", bufs=2) as wpool,
                ):
                    def load_wk(cb):
                        wt = wpool.tile([128, NKO, 128], bf16, tag="wt",
                                        name=f"wk{cb}")
                        d = nc.sync.dma_start(wt[:], wkp_d[cb])
                        return wt, d

                    with (
                        tc.tile_pool(name="wtv", bufs=2) as wvpool,
                        tc.tile_pool(name="psv", bufs=8, space="PSUM") as psvpool,
                    ):
                        banks = [
                            psvpool.tile([128, VC], fp32, tag="psv",
                                         name=f"psv{tb}")
                            for tb in range(8)
                        ]
                        wvcs = []
                        # consumption-ordered startup stream: level ch is
                        # {wv_ch, x_ch} in parallel, after level ch-1.
                        # Chunk 0 is split into two half-levels so the first
                        # v matmuls start ~4 us earlier.
                        for ch in range(NCH):
                            wvc = wvpool.tile([128, KOC, VC], bf16, tag="wv",
                                              name=f"wv{ch}")
                            if ch == 0:
                                # quarter-granularity levels: the PE's first
                                # matmuls start behind a 0.6 MiB bite
                                for q in range(4):
                                    qs = slice(2 * q, 2 * q + 2)
                                    chain(nc.sync.dma_start(
                                              wvc[:, qs, :],
                                              wvp_d[0][:, qs, :]),
                                          nc.sync.dma_start(
                                              xts[0][:, qs, :],
                                              xp_d[0][:, qs, :]))
                            else:
                                chain(nc.sync.dma_start(wvc[:], wvp_d[ch]),
                                      nc.sync.dma_start(xts[ch][:], xp_d[ch]))
                            wvcs.append(wvc)
                        for ch in range(NCH):
                            if ch < NCH - 1:
                                for j in range(KOC):
                                    for tb in range(8):
                                        nc.tensor.matmul(
                                            banks[tb][:],
                                            xts[ch][:, j,
                                                    tb * 128:(tb + 1) * 128],
                                            wvcs[ch][:, j, :],
                                            start=(ch == 0 and j == 0),
                                            stop=False,
                                        )
                            else:
                                # last chunk: tb-major so each bank finishes
                                # (and evicts) 8 matmuls before the next —
                                # spreads the 8 DVE evictions instead of
                                # bunching them after the final matmul
                                for tb in range(8):
                                    for j in range(KOC):
                                        nc.tensor.matmul(
                                            banks[tb][:],
                                            xts[ch][:, j,
                                                    tb * 128:(tb + 1) * 128],
                                            wvcs[ch][:, j, :],
                                            start=False,
                                            stop=(j == KOC - 1),
                                        )
                                    nc.vector.tensor_copy(v[:, tb, :],
                                                          banks[tb][:])

                    # ---- k projections (4 blocks) ----
                    mapk = {}
                    map_dmas = []
                    for nm, dram in (("Ak", ak_d), ("Bk", bk_d)):
                        mt = mpoolk.tile([128, S], fp32, tag=nm, name=nm)
                        map_dmas.append(nc.sync.dma_start(mt[:], dram[:]))
                        mapk[nm] = mt
                    with (
                        tc.tile_pool(name="evk", bufs=2) as epool,
                        tc.tile_pool(name="psk", bufs=3, space="PSUM") as pspool,
                    ):
                        wk0 = load_wk(0)
                        wk1 = load_wk(1)
                        chain(*map_dmas, wk0[1], wk1[1])
                        pending_wk = [wk0[0], wk1[0]]
                        for cb in range(4):
                            wt = pending_wk.pop(0)
                            psA, psB = proj_block(wt, pspool)
                            if cb + 2 < 4:
                                pending_wk.append(load_wk(cb + 2)[0])
                            for th, ps in ((0, psA), (1, psB)):
                                ts_ = slice(th * 512, th * 512 + 512)
                                rope_evict(epool, ps, mapk["Ak"], mapk["Bk"],
                                           kT[:, cb, ts_], th)

                # ---- q blocks softwarepipelined with attention ----
                with (
                    tc.tile_pool(name="mapsq", bufs=1) as mpoolq,
                    tc.tile_pool(name="wtq", bufs=2) as wqpool,
                    tc.tile_pool(name="evq", bufs=2) as epool,
                    tc.tile_pool(name="qbuf", bufs=2) as qpool,
                    tc.tile_pool(name="ex", bufs=2) as expool,
                    tc.tile_pool(name="sm", bufs=2) as smpool,
                    tc.tile_pool(name="psq", bufs=2, space="PSUM") as psqpool,
                    tc.tile_pool(name="pssc", bufs=4, space="PSUM") as pssc,
                    tc.tile_pool(name="psden", bufs=1, space="PSUM") as psden,
                    tc.tile_pool(name="pso", bufs=1, space="PSUM") as psopool,
                ):
                    mapq = {}
                    mq_dmas = []
                    for nm, dram in (("Aq", aq_d), ("Bq", bq_d)):
                        mt = mpoolq.tile([128, S], fp32, tag=nm, name=nm)
                        mq_dmas.append(nc.sync.dma_start(mt[:], dram[:]))
                        mapq[nm] = mt

                    wq_next = [None]
                    wq0_dma = []

                    def load_wq(cb):
                        wt = wqpool.tile([128, NKO, 128], bf16, tag="wt",
                                         name=f"wq{cb}")
                        d = nc.sync.dma_start(wt[:], wqp_d[cb])
                        if cb == 0:
                            wq0_dma.append(d)
                        return wt

                    wq_next[0] = load_wq(0)
                    chain(*mq_dmas, wq0_dma[0])

                    def make_scores(cb, qt):
                        """16 closures: scores MM + exp for (cb, sh, tb).
                        Returns (closures, expT tiles per sh)."""
                        h = cb // 4
                        exps = [
                            expool.tile([128, 8, 512], bf16, tag=f"expT{sh}",
                                        name=f"expT{cb}_{sh}")
                            for sh in range(2)
                        ]
                        closures = []
                        for sh in range(2):
                            ss = slice(sh * 512, sh * 512 + 512)
                            for tb in range(8):
                                def emit(sh=sh, ss=ss, tb=tb):
                                    psc = pssc.tile([128, 512], fp32,
                                                    tag="psc", name="psc")
                                    nc.tensor.matmul(
                                        psc[:],
                                        kT[:, h, tb * 128:(tb + 1) * 128],
                                        qt[:, ss],
                                        start=True, stop=True,
                                    )
                                    nc.scalar.activation(
                                        exps[sh][:, tb], psc[:],
                                        mybir.ActivationFunctionType.Exp,
                                    )
                                closures.append(emit)
                        return closures, exps

                    def emit_tails(cb, exps, interleave):
                        """softmax denom + attn@v for head-block cb."""
                        h = cb // 4
                        for sh in range(2):
                            ss = slice(sh * 512, sh * 512 + 512)
                            expT = exps[sh]
                            # denom: DVE tree-add over tb (saves 7 PE matmuls),
                            # then one ones-matmul for the partition sum
                            tt = []
                            for i in range(4):
                                t = smpool.tile([128, 512], fp16, tag=f"ta{i}",
                                                name=f"ta{i}")
                                nc.vector.tensor_add(t[:], expT[:, 2 * i],
                                                     expT[:, 2 * i + 1])
                                tt.append(t)
                                if interleave:
                                    interleave.pop(0)()
                            nc.vector.tensor_add(tt[0][:], tt[0][:], tt[1][:])
                            nc.vector.tensor_add(tt[2][:], tt[2][:], tt[3][:])
                            nc.vector.tensor_add(tt[0][:], tt[0][:], tt[2][:])
                            pden = psden.tile([128, 512], fp32, tag="pd",
                                              name="pd")
                            nc.tensor.matmul(pden[:], ones_h[:], tt[0][:],
                                             start=True, stop=True)
                            if interleave:
                                interleave.pop(0)()
                            invb = smpool.tile([128, 512], fp32, tag="invb",
                                               name="invb")
                            nc.vector.reciprocal_approx_fast(invb[:], pden[:])
                            po = psopool.tile([128, 512], fp32, tag="po",
                                              name="po")
                            for tb in range(8):
                                nc.tensor.matmul(
                                    po[:],
                                    v[:, tb, h * 128:(h + 1) * 128],
                                    expT[:, tb],
                                    start=(tb == 0), stop=(tb == 7),
                                )
                                if interleave and tb % 2 == 1:
                                    interleave.pop(0)()
                            nc.vector.tensor_mul(oT[:, cb, ss], po[:], invb[:])

                    pending_scores = []
                    pending_exps = None
                    for cb in range(16):
                        wt = wq_next[0]
                        psA, psB = proj_block(wt, psqpool,
                                              interleave=pending_scores,
                                              early=(cb == 1))
                        if cb + 1 < 16:
                            wq_next[0] = load_wq(cb + 1)
                        if cb == 14:
                            # prefetch first epilogue wo strip into the slot
                            # that proj(14) just freed
                            woe0 = wqpool.tile([128, 8, 512], bf16, tag="wt",
                                               name="woe0")
                            nc.sync.dma_start(woe0[:], wop_d[0, 0])
                        qt = qpool.tile([128, S], bf16, tag="qt", name="qt")
                        for th, ps in ((0, psA), (1, psB)):
                            ts_ = slice(th * 512, th * 512 + 512)
                            rope_evict(epool, ps, mapq["Aq"], mapq["Bq"],
                                       qt[:, ts_], th)
                        if cb > 0:
                            emit_tails(cb - 1, pending_exps, pending_scores)
                        assert not pending_scores
                        pending_scores, pending_exps = make_scores(cb, qt)

                    # epilogue: last head's scores, then two early phase-3
                    # output groups (hh=0, tb=0/1) on recycled pools fill the
                    # PE while ACT finishes the last head's exp stream; their
                    # co=15 step (needs oT[:,15] from tails(15)) comes after.
                    for c in pending_scores:
                        c()
                    woe1 = wqpool.tile([128, 8, 512], bf16, tag="wt",
                                       name="woe1")
                    nc.sync.dma_start(woe1[:], wop_d[0, 1])
                    woe = [woe0, woe1]
                    egs = [
                        psqpool.tile([128, 512], fp32, tag="ps", name=f"eg{tb}")
                        for tb in range(2)
                    ]
                    # co 0..7 (strip 0, already resident) for both groups
                    # while strip 1 loads; then co 8..14
                    for tb in range(2):
                        for co in range(8):
                            nc.tensor.matmul(
                                egs[tb][:], oT[:, co, tb * 128:(tb + 1) * 128],
                                woe[0][:, co, :],
                                start=(co == 0), stop=False,
                            )
                    for tb in range(2):
                        for co in range(8, 15):
                            nc.tensor.matmul(
                                egs[tb][:], oT[:, co, tb * 128:(tb + 1) * 128],
                                woe[1][:, co - 8, :],
                                start=False, stop=False,
                            )
                    emit_tails(15, pending_exps, [])
                    for tb in range(2):
                        nc.tensor.matmul(
                            egs[tb][:], oT[:, 15, tb * 128:(tb + 1) * 128],
                            woe[1][:, 7, :], start=False, stop=True,
                        )
                        et = epool.tile([128, 512], fp32, tag="t1", name="et")
                        nc.vector.tensor_copy(et[:], egs[tb][:])
                        nc.sync.dma_start(out_r[tb, :, 0:512], et[:])

                # ---- phase 3: out = oT.T @ wo ----
                # hh processed in pairs: each oT stationary-operand load
                # serves two matmuls (halves the LDWEIGHTS stream on HW).
                # wo strips live in the DEAD x tiles (xpool, outer scope):
                # their DMAs' WAR deps are proj(15)'s reads, so they start
                # ~13 us before the attention pools drain — no cold-start
                # stall waiting for freed SBUF addresses. Strip (hh, half)
                # of pair hp sits at xts[2*(hp%2) + (hh-2*hp)][:, 4*half:].
                with (
                    tc.tile_pool(name="outp", bufs=2) as outpool,
                    tc.tile_pool(name="psout", bufs=4, space="PSUM") as psout,
                ):
                    def load_strip_pair(hp):
                        TA = xts[2 * (hp % 2)]
                        TB = xts[2 * (hp % 2) + 1]
                        for half in range(2):
                            srcA = wop_d[2 * hp, half].rearrange(
                                "p (a b) c -> p a (b c)", b=2)
                            nc.sync.dma_start(
                                TA[:, 4 * half:4 * half + 4, :], srcA)
                            srcB = wop_d[2 * hp + 1, half].rearrange(
                                "p (a b) c -> p a (b c)", b=2)
                            nc.sync.dma_start(
                                TB[:, 4 * half:4 * half + 4, :], srcB)
                        return TA, TB

                    def strip_rhs(T, half, co8):
                        # strip flat offset co8*512 within the half's 4-ko
                        # region of the x tile
                        return T[:, 4 * half + co8 // 2,
                                 (co8 % 2) * 512:(co8 % 2) * 512 + 512]

                    tiles = {0: load_strip_pair(0), 1: load_strip_pair(1)}
                    for hp in range(4):
                        hh1 = 2 * hp + 1
                        TA, TB = tiles.pop(hp)
                        for tb in range(8):
                            # (hh=0, tb=0/1) were done early in the epilogue
                            skipA = hp == 0 and tb < 2
                            psoA = None if skipA else psout.tile(
                                [128, 512], fp32, tag="pso", name="psoA")
                            psoB = psout.tile([128, 512], fp32, tag="pso",
                                              name="psoB")
                            for co in range(16):
                                lhsT = oT[:, co, tb * 128:(tb + 1) * 128]
                                if not skipA:
                                    nc.tensor.matmul(
                                        psoA[:], lhsT,
                                        strip_rhs(TA, co // 8, co % 8),
                                        start=(co == 0), stop=(co == 15),
                                    )
                                nc.tensor.matmul(
                                    psoB[:], lhsT,
                                    strip_rhs(TB, co // 8, co % 8),
                                    start=(co == 0), stop=(co == 15),
                                )
                            targets = ((hh1, psoB),) if skipA else (
                                (2 * hp, psoA), (hh1, psoB))
                            for hh, pso_ in targets:
                                ot = outpool.tile([128, 512], fp32, tag="ot",
                                                  name="ot")
                                nc.vector.tensor_copy(ot[:], pso_[:])
                                nc.sync.dma_start(
                                    out_r[tb, :, hh * 512:(hh + 1) * 512],
                                    ot[:])
                        # prefetch pair hp+2 into the tiles hp just finished
                        # reading (emitted after all of hp's matmuls so the
                        # WAR ordering is correct)
                        if hp < 2:
                            tiles[hp + 2] = load_strip_pair(hp + 2)

    nc.compile()
    return nc


def _host_prep(x, wq, wk, wv, wo, start_pos):
    import ml_dtypes

    bf16 = ml_dtypes.bfloat16
    x = np.asarray(x, dtype=np.float32)
    wq = np.asarray(wq, dtype=np.float32)
    wk = np.asarray(wk, dtype=np.float32)
    wv = np.asarray(wv, dtype=np.float32)
    wo = np.asarray(wo, dtype=np.float32)
    sp = int(np.asarray(start_pos))

    perm = np.concatenate([np.arange(0, 128, 2), np.arange(1, 128, 2)])

    def pack_proj(w):
        # w: [H, C] -> [C/128, 128p, NKO, 128c] with rope perm on cols
        C = w.shape[1]
        r = w.reshape(NKO, 128, C // 128, 128)[:, :, :, perm]
        return np.ascontiguousarray(r.transpose(2, 1, 0, 3)).astype(bf16)

    def pack_v(w):
        # w: [H, VC] -> [NCH, 128p, KOC, VC] (no perm)
        r = w.reshape(NCH, KOC, 128, VC)
        return np.ascontiguousarray(r.transpose(0, 2, 1, 3)).astype(bf16)

    def pack_wo(w):
        # w: [COH, H] -> [8hh, 2half, 128p, 8co, 512]; wo row =
        # half*1024 + co*128 + p, col = hh*512 + c
        r = w.reshape(2, 8, 128, 8, 512)
        return np.ascontiguousarray(r.transpose(3, 0, 2, 1, 4)).astype(bf16)

    def pack_x(xb):
        # xb: [S, H] -> xT[H, S] -> [NCH, 128p, KOC, S]
        xT = np.ascontiguousarray(xb.T).reshape(NCH, KOC, 128, S)
        return np.ascontiguousarray(xT.transpose(0, 2, 1, 3)).astype(bf16)

    inv_freq = 1.0 / (ROPE_BASE ** (np.arange(0, D, 2, dtype=np.float32) / D))
    t = np.arange(sp, sp + S, dtype=np.float32)
    freqs = t[None, :] * inv_freq[:, None]  # [64, S]
    sin, cos = np.sin(freqs), np.cos(freqs)
    A = np.concatenate([sin, sin], axis=0).astype(np.float32)  # [128, S]
    Bm = np.concatenate([-cos, cos], axis=0).astype(np.float32)
    scale = np.float32(1.0 / np.sqrt(np.float32(D)))
    maps = {
        "ropeAq": np.ascontiguousarray(A * scale),
        "ropeBq": np.ascontiguousarray(Bm * scale),
        "ropeAk": np.ascontiguousarray(A),
        "ropeBk": np.ascontiguousarray(Bm),
    }

    # weights are shared across batches: pack once per tp half
    wpacks = []
    for j in range(2):
        wpacks.append({
            "wqp": pack_proj(wq[:, j * QC:(j + 1) * QC]),
            "wkp": pack_proj(wk[:, j * KC:(j + 1) * KC]),
            "wvp": pack_v(wv[:, j * VC:(j + 1) * VC]),
            "wop": pack_wo(wo[j * COH:(j + 1) * COH, :]),
        })
    xpacks = [pack_x(x[b]) for b in range(B)]

    in_maps = []
    for c in range(NCORES):
        b, j = divmod(c, 2)
        im = {"xp": xpacks[b]}
        im.update(wpacks[j])
        im.update(maps)
        in_maps.append(im)
    return in_maps


def kernel(x, wq, wk, wv, wo, start_pos=0, _trace=False):
    from concourse.bass_utils import run_bass_kernel_spmd

    if "nc" not in _CACHE:
        _CACHE["nc"] = _build()
    nc = _CACHE["nc"]

    in_maps = _host_prep(x, wq, wk, wv, wo, start_pos)
    res = run_bass_kernel_spmd(nc, in_maps, core_ids=list(range(NCORES)), trace=_trace)
    _CACHE["last_result"] = res

    out = np.empty((B, S, H), dtype=np.float32)
    for b in range(B):
        out[b] = res.results[2 * b]["out"] + res.results[2 * b + 1]["out"]
    return out
